# revision 1
# baseline (speedup 1.0000x reference)
"""CCNet cross-layer criss-cross attention, distributed across Trainium2
NeuronCores.

Sharding: data-parallel over batch (B=4). The criss-cross attention is
independent per batch element, so batch sharding is lossless and needs no
cross-core communication; the small 1x1 conv weights are replicated to
every core. Each core runs the full per-batch pipeline (bilinear 2x
upsample, query/value 1x1 convs, 2 rounds of criss-cross attention, fused
1x1 conv, eval-mode BatchNorm + ReLU) compiled for the NeuronCore; the
host gathers the per-batch outputs back into the full [4,256,128,128]
tensor.
"""
import numpy as np
import jax
import jax.numpy as jnp
from functools import partial

NEG_INF = -1e9
BN_EPS = 1e-5


def _conv1x1(x, W, b=None):
    # x: [B,Ci,H,W], W: [Co,Ci]
    y = jnp.einsum('oi,bihw->bohw', W, x)
    if b is not None:
        y = y + b[None, :, None, None]
    return y


def _criss_cross(q, k, v, Wq, bq, Wk, bk, Wv, bv, gamma):
    B, C, H, W = q.shape
    pq = _conv1x1(q, Wq, bq)
    pk = _conv1x1(k, Wk, bk)
    pv = _conv1x1(v, Wv, bv)
    eH = jnp.einsum('bchw,bcxw->bhwx', pq, pk)
    eH = eH + NEG_INF * jnp.eye(H, dtype=q.dtype)[:, None, :]
    eW = jnp.einsum('bchw,bchx->bhwx', pq, pk)
    att = jax.nn.softmax(jnp.concatenate([eH, eW], axis=-1), axis=-1)
    attH, attW = att[..., :H], att[..., H:]
    outH = jnp.einsum('bcxw,bhwx->bchw', pv, attH)
    outW = jnp.einsum('bchx,bhwx->bchw', pv, attW)
    return gamma * (outH + outW) + v


def _forward(low_feature, high_feature, Wc1, bc1, Wc2, bc2, Wq, bq, Wk, bk,
             Wv, bv, gamma, Wb, bn_gamma, bn_beta, bn_mean, bn_var):
    B, C, Hl, Wl = low_feature.shape
    high_up = jax.image.resize(high_feature, (B, C, Hl, Wl), method='linear')
    query = _conv1x1(jnp.concatenate([high_up, low_feature], axis=1), Wc1, bc1)
    value = _conv1x1(high_up, Wc2, bc2)
    for _ in range(2):
        value = _criss_cross(query, value, value, Wq, bq, Wk, bk, Wv, bv, gamma)
    fused = _conv1x1(jnp.concatenate([value, high_up], axis=1), Wb)
    inv = jax.lax.rsqrt(bn_var + BN_EPS)
    y = (fused - bn_mean[None, :, None, None]) * (inv * bn_gamma)[None, :, None, None] \
        + bn_beta[None, :, None, None]
    return jax.nn.relu(y)


_compiled = None


def _get_compiled():
    global _compiled
    if _compiled is None:
        n_batch_shards = 4  # B=4, one batch element per core
        devices = jax.devices()[:n_batch_shards]
        _compiled = jax.pmap(
            _forward,
            in_axes=(0, 0) + (None,) * 16,
            devices=devices,
        )
    return _compiled


def kernel(low_feature, high_feature, Wc1, bc1, Wc2, bc2, Wq, bq, Wk, bk,
           Wv, bv, gamma, Wb, bn_gamma, bn_beta, bn_mean, bn_var):
    fn = _get_compiled()
    B = low_feature.shape[0]
    # one batch element per core: [4,...] -> [4,1,...] shards
    lf = np.ascontiguousarray(low_feature, dtype=np.float32).reshape(
        (B, 1) + low_feature.shape[1:])
    hf = np.ascontiguousarray(high_feature, dtype=np.float32).reshape(
        (B, 1) + high_feature.shape[1:])
    out = fn(lf, hf, Wc1, bc1, Wc2, bc2, Wq, bq, Wk, bk, Wv, bv,
             jnp.float32(gamma), Wb, bn_gamma, bn_beta, bn_mean, bn_var)
    out = np.asarray(out)  # [4,1,256,128,128]
    return np.ascontiguousarray(out.reshape((B,) + out.shape[2:]), dtype=np.float32)

